# revision 1
# baseline (speedup 1.0000x reference)
"""Trainium2 Bass kernel for nn_CONVClassifier (embedding -> pair-conv -> maxpool
-> sigmoid -> 2-layer classifier -> log_softmax).

Sharding: data-parallel over batch. 64 sequences / 8 cores = 8 sequences per core.
Weights replicated; each core gets a host-compacted per-core embedding table
(<=2048 unique tokens per core), so the device gather uses the TIE-accelerated
dma_gather with int16 indices and hardware transpose.

Per-core pipeline:
  1. dma_gather(transpose=True) pulls the 4 overlapping position segments
     (640 tokens each) straight into embT layout [e_chunk, pos] - the DMA
     xbar does the transpose, no PE/DVE transpose work at all.
  2. conv[s, pos] = Wc_left.T @ emb[pos] + Wc_right.T @ emb[pos+1] as K=1024
     PSUM accumulation, using a free-dim offset for the +1 shift. fp16
     operands (1 cycle/col, single LDWEIGHTS), fp32 accumulate.
  3. reduce_max over valid positions per sequence -> sigmoid(max + bc).
  4. Classifier S->50->2 and 2-class log_softmax via ln(1+exp(d)), fp32.
"""

import numpy as np
from contextlib import ExitStack

import concourse.bass as bass
import concourse.tile as tile
from concourse import bacc, mybir
from concourse.bass_utils import run_bass_kernel_spmd
from concourse.tile import add_dep_helper

# Problem shapes (hardcoded per harness contract).
V, E, S, NCLASS = 50000, 512, 1024, 2
B, L = 64, 256
NCORES = 8
BLOC = B // NCORES          # 8 sequences per core
POS = BLOC * L              # 2048 positions per core
PAIRS = BLOC // 2           # 4 sequence-pairs (N=512 per matmul group)
SEG = 640                   # gathered positions per segment (512 + shift + pad)
SC = S // 128               # 8 output-channel chunks
KC = (2 * E) // 128         # 8 contraction chunks (f = 2E = 1024)
EC = E // 128               # 4 embedding chunks
UT = 2048                   # compacted per-core table rows (>= unique tokens)

F32 = mybir.dt.float32
F16 = mybir.dt.float16
I16 = mybir.dt.int16
AF = mybir.ActivationFunctionType

_CACHE = {}


def build_program():
    nc = bacc.Bacc("TRN2", target_bir_lowering=False, debug=False,
                   num_devices=NCORES, enable_partition_id=False)

    table = nc.dram_tensor("table", [UT, E], F16, kind="ExternalInput")
    ind = nc.dram_tensor("ind", [128, PAIRS, SEG // 16], I16,
                         kind="ExternalInput")
    wct = nc.dram_tensor("wct", [128, SC, KC, 128], F16, kind="ExternalInput")
    w1t = nc.dram_tensor("w1t", [128, SC, 50], F32, kind="ExternalInput")
    bct = nc.dram_tensor("bct", [128, SC], F32, kind="ExternalInput")
    b1c = nc.dram_tensor("b1c", [50, 1], F32, kind="ExternalInput")
    w2z = nc.dram_tensor("w2z", [51, NCLASS], F32, kind="ExternalInput")
    ident_d = nc.dram_tensor("ident", [128, 128], F16, kind="ExternalInput")
    ind32 = nc.dram_tensor("ind32", [128, 5], mybir.dt.int32,
                           kind="ExternalInput")
    out_d = nc.dram_tensor("out", [BLOC, NCLASS], F32, kind="ExternalOutput")

    with tile.TileContext(nc) as tc, ExitStack() as ctx:
        const = ctx.enter_context(tc.tile_pool(name="const", bufs=1))
        rawp = ctx.enter_context(tc.tile_pool(name="rawp", bufs=5))
        trp = ctx.enter_context(
            tc.tile_pool(name="trp", bufs=1, space=bass.MemorySpace.PSUM))
        warmp = ctx.enter_context(
            tc.tile_pool(name="warmp", bufs=1, space=bass.MemorySpace.PSUM))
        mmp = ctx.enter_context(
            tc.tile_pool(name="mmp", bufs=4, space=bass.MemorySpace.PSUM))
        tailp = ctx.enter_context(
            tc.tile_pool(name="tailp", bufs=1, space=bass.MemorySpace.PSUM))

        # --- constants / weights ---
        ind32_sb = const.tile([128, 5], mybir.dt.int32)
        nc.sync.dma_start(ind32_sb[:], ind32[:])
        ind_sb = const.tile([128, PAIRS, SEG // 16], I16)
        nc.sync.dma_start(ind_sb[:], ind[:])
        identity = const.tile([128, 128], F16)
        nc.sync.dma_start(identity[:], ident_d[:])

        wct_sb = const.tile([128, SC, KC, 128], F16)
        for sc in range(SC):
            nc.sync.dma_start(wct_sb[:, sc], wct[:, sc])
        sent_max = [const.tile([128, BLOC], F32, tag=f"smax{sc}",
                                name=f"smax{sc}") for sc in range(SC)]
        sent_sig = [const.tile([128, BLOC], F32, tag=f"ssig{sc}",
                                name=f"ssig{sc}") for sc in range(SC)]

        # --- gather. Segment 0 uses the library-free indirect-DMA +
        # PE-transpose path (can start right after the preamble); segments
        # 1-3 use the transposing dma_gather, which is gated ~12us by the
        # GPSIMD ucode library load but lands [e, pos] directly. ---
        embsegs = []
        seg0 = const.tile([128, EC, SEG], F16, tag="seg0", name="seg0")
        embsegs.append(seg0)
        raws = []
        last_gather = None
        for t in range(5):
            raw = rawp.tile([128, E], F16, tag="raw", name=f"raw{t}")
            gi = nc.gpsimd.indirect_dma_start(
                out=raw[:], out_offset=None, in_=table[:],
                in_offset=bass.IndirectOffsetOnAxis(
                    ap=ind32_sb[:, t:t + 1], axis=0))
            raws.append(raw)
            last_gather = gi
        for p in range(1, PAIRS):
            seg = const.tile([128, EC, SEG], F16, tag=f"seg{p}",
                             name=f"seg{p}")
            gi = nc.gpsimd.dma_gather(
                out_ap=seg[:], in_ap=table[:], idxs_ap=ind_sb[:, p, :],
                num_idxs=SEG, num_idxs_reg=SEG, elem_size=E, transpose=True)
            # Keep the serial Q7 descriptor generator in consumption order:
            # the library-free indirect gathers (segment 0) must run first.
            add_dep_helper(gi.ins, last_gather.ins, sync=False,
                           reason="q7 order: seg0 indirects before dma_gather")
            last_gather = gi
            embsegs.append(seg)
        # PE warmup: keep the tensor engine busy during the DMA prologue so
        # the HAM clock gate is at 8/8 when the real stream starts.
        warm = warmp.tile([128, 2 * L], F32, tag="warm")

        def warmup(n):
            for w in range(n):
                nc.tensor.matmul(warm[:], lhsT=wct_sb[:, 0, w % KC, :],
                                 rhs=wct_sb[:, 0, 0:EC, :],
                                 start=True, stop=True)

        warmup(16)

        def transpose_tile(t):
            tr = trp.tile([128, EC, 128], F16, tag="tr")
            for cc in range(EC):
                nc.tensor.transpose(out=tr[:, cc, :],
                                    in_=raws[t][:, cc * 128:(cc + 1) * 128],
                                    identity=identity[:])
            nc.vector.tensor_copy(out=seg0[:, :, t * 128:(t + 1) * 128],
                                  in_=tr[:])

        for t in range(4):
            transpose_tile(t)
            warmup(3)

        def do_tile4():
            transpose_tile(4)
            warmup(2)

        w1t_sb = const.tile([128, SC, 50], F32)
        nc.sync.dma_start(w1t_sb[:], w1t[:])
        bct_sb = const.tile([128, SC], F32)
        nc.sync.dma_start(bct_sb[:], bct[:])
        b1_sb = const.tile([50, 1], F32)
        nc.sync.dma_start(b1_sb[:], b1c[:])
        w2z_sb = const.tile([51, NCLASS], F32)
        nc.sync.dma_start(w2z_sb[:], w2z[:])

        def mm_kc(seg, ps, sc, kc):
            if kc < EC:
                rhs = seg[:, kc, 0:2 * L]
            else:
                rhs = seg[:, kc - EC, 1:2 * L + 1]
            nc.tensor.matmul(ps[:], lhsT=wct_sb[:, sc, kc, :], rhs=rhs,
                             start=(kc == 0), stop=(kc == KC - 1))

        def finish_sc(p, sc, ps):
            nc.vector.tensor_reduce(
                out=sent_max[sc][:, 2 * p:2 * p + 2],
                in_=ps[:].rearrange("q (h l) -> q h l", h=2)[:, :, 0:L - 1],
                axis=mybir.AxisListType.X, op=mybir.AluOpType.max)
            if p == PAIRS - 1:
                # sent = sigmoid(max + bc); fires as soon as this channel
                # chunk's last reduce lands, overlapping the main stream.
                nc.scalar.activation(out=sent_sig[sc][:],
                                     in_=sent_max[sc][:],
                                     func=AF.Sigmoid,
                                     bias=bct_sb[:, sc:sc + 1])

        def do_pair(p):
            seg = embsegs[p]
            for sc in range(SC):
                ps = mmp.tile([128, 2 * L], F32, tag="mm")
                for kc in range(KC):
                    mm_kc(seg, ps, sc, kc)
                finish_sc(p, sc, ps)

        # Pair 0 is split: its kc<4 matmuls only read tiles 0-3 of seg0, so
        # they run while tile 4 (needed by the +1-shifted kc>=4 reads) is
        # still being gathered/transposed.
        ps0 = [mmp.tile([128, 2 * L], F32, tag="mm", name=f"ps0_{sc}")
               for sc in range(4)]
        for sc in range(4):
            for kc in range(EC):
                mm_kc(embsegs[0], ps0[sc], sc, kc)
        do_tile4()
        for sc in range(4):
            for kc in range(EC, KC):
                mm_kc(embsegs[0], ps0[sc], sc, kc)
            finish_sc(0, sc, ps0[sc])
        for sc in range(4, SC):
            ps = mmp.tile([128, 2 * L], F32, tag="mm")
            for kc in range(KC):
                mm_kc(embsegs[0], ps, sc, kc)
            finish_sc(0, sc, ps)
        for p in range(1, PAIRS):
            do_pair(p)

        # h = W1 @ sent + b1  -> [50, b]
        h_ps = tailp.tile([50, BLOC], F32, tag="hps")
        for sc in range(SC):
            nc.tensor.matmul(h_ps[:], lhsT=w1t_sb[:, sc, :],
                             rhs=sent_sig[sc][:],
                             start=(sc == 0), stop=(sc == SC - 1))
        h_sb = const.tile([51, BLOC], F32)
        # start partitions must be 32-aligned: fill 32..50 with 1.0 first,
        # the h+b1 write below then restores rows 32..49.
        nc.gpsimd.memset(h_sb[32:51, :], 1.0)
        nc.vector.tensor_scalar_add(h_sb[0:50, :], h_ps[:], b1_sb[:, 0:1])

        # z[b] = (-d, d) with d = (l1 - l0) + (b2[1] - b2[0]) computed in ONE
        # matmul: h is augmented with a constant-1 row and w2z holds the
        # class-difference weights with the bias difference as its last row.
        # Then out_c = ln(sigmoid(z_c)) is exactly 2-class log_softmax; the
        # sigmoid reuses the sent table, only Ln switches ACT tables.
        z_ps = tailp.tile([BLOC, NCLASS], F32, tag="lps")
        nc.tensor.matmul(z_ps[:], lhsT=h_sb[:], rhs=w2z_sb[:],
                         start=True, stop=True)
        sg = const.tile([BLOC, NCLASS], F32)
        nc.scalar.activation(out=sg[:], in_=z_ps[:], func=AF.Sigmoid)
        out_sb = const.tile([BLOC, NCLASS], F32)
        nc.scalar.activation(out=out_sb[:], in_=sg[:], func=AF.Ln)
        nc.sync.dma_start(out_d[:], out_sb[:])

    nc.compile()
    return nc


def _get_program():
    if "nc" not in _CACHE:
        _CACHE["nc"] = build_program()
    return _CACHE["nc"]


def prepare_in_maps(inputs):
    inp = {k: np.asarray(v) for k, v in inputs.items()}
    idx = inp["inputs"].astype(np.int64)                       # [64, 256]
    table16 = np.asarray(inp["emb_table"], dtype=np.float16)   # [V, E]
    Wc = np.asarray(inp["Wc"], dtype=np.float32)               # [S, 2E]
    bc = np.asarray(inp["bc"], dtype=np.float32)
    W1 = np.asarray(inp["W1"], dtype=np.float32)               # [50, S]
    b1 = np.asarray(inp["b1"], dtype=np.float32)
    W2 = np.asarray(inp["W2"], dtype=np.float32)               # [2, 50]
    b2 = np.asarray(inp["b2"], dtype=np.float32)

    # wct[p, sc, kc, j] = Wc[sc*128+j, kc*128+p]
    wct = np.ascontiguousarray(
        Wc.reshape(SC, 128, KC, 128).transpose(3, 0, 2, 1)).astype(np.float16)
    # w1t[p, sc, j] = W1[j, sc*128+p]
    w1t = np.ascontiguousarray(W1.reshape(50, SC, 128).transpose(2, 1, 0))
    bctm = np.ascontiguousarray(bc.reshape(SC, 128).T)         # [128, SC]
    b1c = np.ascontiguousarray(b1.reshape(50, 1))
    b2dv = float(b2[1]) - float(b2[0])
    w2zm = np.zeros((51, NCLASS), dtype=np.float32)
    w2zm[0:50, 0] = W2[0] - W2[1]
    w2zm[0:50, 1] = W2[1] - W2[0]
    w2zm[50, 0] = -b2dv
    w2zm[50, 1] = b2dv

    in_maps = []
    for c in range(NCORES):
        flat = idx[c * BLOC:(c + 1) * BLOC].reshape(-1)        # [2048]
        # Compact the table to this core's unique tokens so row ids fit int16.
        uniq, inv = np.unique(flat, return_inverse=True)
        assert len(uniq) <= UT
        table_c = np.zeros((UT, E), dtype=np.float16)
        table_c[:len(uniq)] = table16[uniq]
        inv16 = inv.astype(np.int16)                           # [2048]
        # Per-segment indices: seg p covers positions 512p .. 512p+SEG (0-pad).
        ind16 = np.zeros((128, PAIRS, SEG // 16), dtype=np.int16)
        for p in range(PAIRS):
            segidx = np.zeros(SEG, dtype=np.int16)
            n = min(SEG, POS - 512 * p)
            segidx[:n] = inv16[512 * p:512 * p + n]
            blk = segidx.reshape(SEG // 16, 16).T              # [16, SEG//16]
            ind16[:, p, :] = np.tile(blk, (8, 1))              # replicate x8
        ind32m = np.ascontiguousarray(
            inv16[:640].astype(np.int32).reshape(5, 128).T)
        in_maps.append({"table": table_c, "ind": ind16, "wct": wct,
                        "w1t": w1t, "bct": bctm, "b1c": b1c,
                        "w2z": w2zm,
                        "ident": np.eye(128, dtype=np.float16),
                        "ind32": ind32m})
    return in_maps


def run(inputs, trace=False):
    nc = _get_program()
    in_maps = prepare_in_maps(inputs)
    res = run_bass_kernel_spmd(nc, in_maps, list(range(NCORES)), trace=trace)
    out = np.concatenate(
        [res.results[c]["out"] for c in range(NCORES)], axis=0)
    return out.astype(np.float32), res


def kernel(**inputs) -> np.ndarray:
    out, _ = run(inputs, trace=False)
    return out

